# revision 20
# baseline (speedup 1.0000x reference)
# kernel.py — self-contained Trainium2 Bass kernel for nn_AttnReadout.
# Sharding: graph-level data parallel. Device d gets 512 contiguous graphs
# (131072 nodes). BN stats via per-device partial sums + AllReduce.
# sigmoid(y) computed as 0.5 + 0.5*tanh(y/2) so the whole inner loop stays
# on one ACT table set (tanh+exp coexist in exp_and_others).
#
# fp16 data path: feat is shipped and streamed as fp16, which halves both
# host->device transfer and HBM traffic and runs the PE matmul streams at
# 1 cycle/row (fp32 runs at 4). BN statistics, the attention softmax and
# the output tail accumulate in fp32 (PSUM); measured end-to-end relative
# error vs the fp32 reference is ~3e-4.
#
# DMA layout: feat tiles are loaded "pair-interleaved" — partition p holds
# nodes (2p, 2p+1) of a 256-node window — so every DMA descriptor moves a
# 512-byte contiguous run (two 256B feature rows). 256B runs pay a 2x DMA
# latency penalty on TRN2; this layout runs at full HBM bandwidth. The
# within-graph node permutation is harmless: every per-node quantity is
# computed column-consistently and graphs align with 256-node windows.
import os
import sys

sys.path.insert(0, "/opt/trn_rl_repo")
os.environ.setdefault("JAX_PLATFORMS", "axon")

import numpy as np

NUM_GRAPHS = 4096
NODES_PER_GRAPH = 256
N_TOTAL = NUM_GRAPHS * NODES_PER_GRAPH
IN_DIM = 128
HID_DIM = 128
OUT_DIM = 256
BN_EPS = 1e-5
N_CORES = 8

G_CORE = NUM_GRAPHS // N_CORES            # 512 graphs
N_CORE = G_CORE * NODES_PER_GRAPH         # 131072 nodes
CHUNK = 128
PCH_GRP = 16                               # pchunks (=graphs) per DMA group
GRP_NODES = PCH_GRP * NODES_PER_GRAPH      # 4096 nodes / group (1 MiB fp16)

_CACHE = {}


def build_nc(n_cores, g_core):
    import concourse.bass as bass
    import concourse.bacc as bacc
    import concourse.tile as tile
    from concourse import mybir
    from concourse.masks import make_identity

    key = (n_cores, g_core)
    if key in _CACHE:
        return _CACHE[key]

    f32 = mybir.dt.float32
    f16 = mybir.dt.float16
    nc = bacc.Bacc("TRN2", target_bir_lowering=False, debug=False,
                   enable_asserts=False, num_devices=n_cores)
    n_core = g_core * NODES_PER_GRAPH
    feat = nc.dram_tensor("feat", [n_core, IN_DIM], f16, kind="ExternalInput")
    flast = nc.dram_tensor("flast", [g_core, IN_DIM], f32, kind="ExternalInput")
    W_u = nc.dram_tensor("W_u", [IN_DIM, HID_DIM], f32, kind="ExternalInput")
    W_v = nc.dram_tensor("W_v", [IN_DIM, HID_DIM], f32, kind="ExternalInput")
    b_v = nc.dram_tensor("b_v", [HID_DIM], f32, kind="ExternalInput")
    w_e = nc.dram_tensor("w_e", [HID_DIM, 1], f32, kind="ExternalInput")
    W_out = nc.dram_tensor("W_out", [IN_DIM, OUT_DIM], f32, kind="ExternalInput")
    gamma = nc.dram_tensor("gamma", [IN_DIM], f32, kind="ExternalInput")
    beta = nc.dram_tensor("beta", [IN_DIM], f32, kind="ExternalInput")
    rst = nc.dram_tensor("rst", [g_core, OUT_DIM], f32, kind="ExternalOutput")

    with tile.TileContext(nc) as tc:
        _emit(nc, tc, bass, tile, mybir, make_identity,
              feat, flast, W_u, W_v, b_v, w_e, W_out, gamma, beta, rst,
              n_cores, g_core)
    nc.compile()
    _CACHE[key] = nc
    return nc


def _emit(nc, tc, bass, tile, mybir, make_identity,
          feat, flast, W_u, W_v, b_v, w_e, W_out, gamma, beta, rst,
          n_cores, g_core):
    from contextlib import ExitStack

    f32 = mybir.dt.float32
    f16 = mybir.dt.float16
    AF = mybir.ActivationFunctionType
    ts = bass.ts
    n_core = g_core * NODES_PER_GRAPH
    n_total = n_core * n_cores
    n_grps = n_core // GRP_NODES              # 32

    # pair-interleaved group view: group ng, partition p, pchunk c, layer q
    # holds node ng*4096 + c*256 + 2p + q. Innermost (q i) = 512B contiguous.
    feat_g = feat[:, :].rearrange("(ng c p q) i -> ng p c q i",
                                  p=CHUNK, c=PCH_GRP, q=2)

    ctx = ExitStack()
    with ctx:
        consts = ctx.enter_context(tc.tile_pool(name="consts", bufs=1))
        ident_h = consts.tile([128, 128], f16, tag="ident_h")
        make_identity(nc, ident_h[:])
        ident_f = consts.tile([128, 128], f32, tag="ident_f")
        make_identity(nc, ident_f[:])
        ones_h = consts.tile([128, 1], f16, tag="ones_h")
        nc.vector.memset(ones_h[:], 1.0)
        ones_col = consts.tile([128, 1], f32, tag="ones_f")
        nc.vector.memset(ones_col[:], 1.0)
        ones_row = consts.tile([1, 128], f32, tag="ones_r")
        nc.vector.memset(ones_row[:], 1.0)

        # indicator for the v-fold matmuls: ind32[q, j] = 1 iff j//256 == q.
        # Built via two affine selects on the idle gpsimd engine so it
        # overlaps the phase-A DMA stream.
        npg = NODES_PER_GRAPH
        ind32 = consts.tile([32, 32 * npg], f16, tag="ind32")
        nc.gpsimd.memset(ind32[:], 1.0)
        nc.gpsimd.affine_select(
            out=ind32[:], in_=ind32[:], compare_op=mybir.AluOpType.is_ge,
            fill=0.0, base=npg - 1, pattern=[[-1, 32 * npg]],
            channel_multiplier=npg)
        nc.gpsimd.affine_select(
            out=ind32[:], in_=ind32[:], compare_op=mybir.AluOpType.is_ge,
            fill=0.0, base=0, pattern=[[1, 32 * npg]],
            channel_multiplier=-npg)

        # ---------------- Phase A: BN stats (fp16 streams) ----------------
        # feat loads go on the gpsimd (Pool) software-DGE queue; the SP
        # hardware queue is reserved for the XBAR transposes in phase B.
        pfeat = ctx.enter_context(tc.tile_pool(name="pfeat", bufs=3))
        with tc.tile_pool(name="pa_sq", bufs=2) as pa_sq, \
             tc.tile_pool(name="pa_ps", bufs=1, space="PSUM") as pa_ps:
            ps_sum = pa_ps.tile([1, 512], f32, tag="sum")
            ps_sq = pa_ps.tile([1, 512], f32, tag="sq")
            for ng in range(n_grps):
                ft = pfeat.tile([128, PCH_GRP, 2, 128], f16)
                nc.sync.dma_start(ft[:], feat_g[ng])
                sq = pa_sq.tile([128, PCH_GRP, 2, 128], f16)
                nc.vector.tensor_mul(sq[:], ft[:], ft[:])
                for j in range(PCH_GRP // 2):
                    first = (ng == 0 and j == 0)
                    last = (ng == n_grps - 1 and j == PCH_GRP // 2 - 1)
                    sl = slice(2 * j, 2 * j + 2)
                    mm1 = nc.tensor.matmul(ps_sum[:], ones_h[:],
                                           ft[:, sl, :, :],
                                           start=first, stop=last,
                                           skip_group_check=True)
                    mm2 = nc.tensor.matmul(ps_sq[:], ones_h[:],
                                           sq[:, sl, :, :],
                                           start=first, stop=last,
                                           skip_group_check=True)
                    # all-ones stationary: let walrus use the 0/1-weight path
                    mm1.is_weight_onezero = True
                    mm2.is_weight_onezero = True
            stats_sb = consts.tile([1, 1024], f32, tag="stats")
            nc.vector.tensor_copy(stats_sb[:, 0:512], ps_sum[:])
            nc.vector.tensor_copy(stats_sb[:, 512:1024], ps_sq[:])

        # ---------------- AllGather of stats + local fold ----------------
        # AllGather costs ~x1.9 less than AllReduce on TRN2 for tiny
        # payloads; the 8-way sum is 2 cheap f32 matmuls against ones.
        gstats = consts.tile([1, 1024], f32, tag="gstats")
        gst_sb = consts.tile([n_cores, 1024], f32, tag="gst_sb")
        if n_cores > 1:
            with tc.tile_pool(name="dram", bufs=1, space="DRAM") as dram:
                cin = dram.tile([1, 1024], f32, tag="cin")
                cout = dram.tile([n_cores, 1024], f32, tag="cout")
                nc.gpsimd.dma_start(cin[:], stats_sb[:])
                nc.gpsimd.collective_compute(
                    "AllGather", mybir.AluOpType.bypass,
                    replica_groups=[list(range(n_cores))],
                    ins=[cin.opt()], outs=[cout.opt()])
                nc.gpsimd.dma_start(gst_sb[:], cout[:, :])
            with tc.tile_pool(name="ag_ps", bufs=1, space="PSUM") as ag_ps:
                agp = ag_ps.tile([1, 1024], f32, tag="agp")
                for h in range(2):
                    nc.tensor.matmul(agp[:, ts(h, 512)],
                                     ones_col[0:n_cores, :],
                                     gst_sb[:, ts(h, 512)],
                                     start=True, stop=True)
                nc.vector.tensor_copy(gstats[:], agp[:])
        else:
            nc.vector.tensor_copy(gstats[:], stats_sb[:])

        # fold 4 sub-chunk partials -> [1,128]; a = gamma*rsqrt(var+eps),
        # b = beta - mean*a
        srow = consts.tile([1, 128], f32, tag="srow")
        qrow = consts.tile([1, 128], f32, tag="qrow")
        t0 = consts.tile([1, 128], f32, tag="t0")
        t1 = consts.tile([1, 128], f32, tag="t1")
        nc.vector.tensor_add(t0[:], gstats[:, 0:128], gstats[:, 128:256])
        nc.vector.tensor_add(t1[:], gstats[:, 256:384], gstats[:, 384:512])
        nc.vector.tensor_add(srow[:], t0[:], t1[:])
        nc.vector.tensor_add(t0[:], gstats[:, 512:640], gstats[:, 640:768])
        nc.vector.tensor_add(t1[:], gstats[:, 768:896], gstats[:, 896:1024])
        nc.vector.tensor_add(qrow[:], t0[:], t1[:])

        mean_r = consts.tile([1, 128], f32, tag="mean")
        ex2_r = consts.tile([1, 128], f32, tag="ex2")
        nc.scalar.mul(mean_r[:], srow[:], 1.0 / n_total)
        nc.scalar.mul(ex2_r[:], qrow[:], 1.0 / n_total)
        var_r = consts.tile([1, 128], f32, tag="var")
        nc.vector.tensor_mul(t0[:], mean_r[:], mean_r[:])
        nc.vector.tensor_scalar_mul(t0[:], t0[:], -1.0)
        nc.vector.tensor_add(var_r[:], t0[:], ex2_r[:])
        eps_t = consts.tile([1, 1], f32, tag="eps")
        nc.vector.memset(eps_t[:], BN_EPS)
        sd_r = consts.tile([1, 128], f32, tag="sd")
        nc.scalar.activation(sd_r[:], var_r[:], AF.Sqrt, bias=eps_t[:], scale=1.0)
        rs_r = consts.tile([1, 128], f32, tag="rs")
        nc.vector.reciprocal(rs_r[:], sd_r[:])

        grow = consts.tile([1, 128], f32, tag="grow")
        brow = consts.tile([1, 128], f32, tag="brow")
        nc.sync.dma_start(grow[:], gamma[:].rearrange("(o p) -> o p", o=1))
        nc.sync.dma_start(brow[:], beta[:].rearrange("(o p) -> o p", o=1))
        a_r = consts.tile([1, 128], f32, tag="a_r")
        b_r = consts.tile([1, 128], f32, tag="b_r")
        nc.vector.tensor_mul(a_r[:], rs_r[:], grow[:])
        nc.vector.tensor_mul(t0[:], mean_r[:], a_r[:])
        nc.vector.tensor_scalar_mul(t0[:], t0[:], -1.0)
        nc.vector.tensor_add(b_r[:], t0[:], brow[:])

        # folded weights + per-graph bias matrix vT (scaled by 0.5 for tanh)
        with tc.tile_pool(name="prep_ps", bufs=1, space="PSUM") as prep_ps, \
             tc.tile_pool(name="flt", bufs=2) as flt_pool:
            aT = consts.tile([128, 1], f32, tag="aT")
            bT = consts.tile([128, 1], f32, tag="bT")
            pT = prep_ps.tile([128, 1], f32, tag="pT")
            nc.tensor.transpose(pT[:], a_r[:], ident_f[0:1, 0:1])
            nc.vector.tensor_copy(aT[:], pT[:])
            pT2 = prep_ps.tile([128, 1], f32, tag="pT2")
            nc.tensor.transpose(pT2[:], b_r[:], ident_f[0:1, 0:1])
            nc.vector.tensor_copy(bT[:], pT2[:])

            Wu_sb = consts.tile([128, HID_DIM], f32, tag="Wu")
            Wv_sb = consts.tile([128, HID_DIM], f32, tag="Wv")
            Wout_sb = consts.tile([128, OUT_DIM], f32, tag="Wout")
            we_sb = consts.tile([128, 1], f32, tag="we")
            bv_col = consts.tile([128, 1], f32, tag="bv")
            nc.sync.dma_start(Wu_sb[:], W_u[:, :])
            nc.sync.dma_start(Wv_sb[:], W_v[:, :])
            nc.sync.dma_start(Wout_sb[:], W_out[:, :])
            nc.sync.dma_start(we_sb[:], w_e[:, :])
            nc.sync.dma_start(bv_col[:], b_v[:].rearrange("(p o) -> p o", o=1))

            Wu_s = consts.tile([128, HID_DIM], f32, tag="Wu_s")
            Wv_s = consts.tile([128, HID_DIM], f32, tag="Wv_s")
            nc.vector.tensor_scalar_mul(Wu_s[:], Wu_sb[:], aT[:])
            nc.vector.tensor_scalar_mul(Wv_s[:], Wv_sb[:], aT[:])
            # fp16 copy for the hot loop
            Wu_h = consts.tile([128, HID_DIM], f16, tag="Wu_h")
            nc.vector.tensor_copy(Wu_h[:], Wu_s[:])

            # we_h = 0.5*w_e (fp16) ; c0b = 0.5*sum(w_e) broadcast column
            we_h = consts.tile([128, 1], f16, tag="we_h")
            nc.scalar.mul(we_h[:], we_sb[:], 0.5)
            c0_ps = prep_ps.tile([1, 1], f32, tag="c0")
            nc.tensor.matmul(c0_ps[:], we_sb[:], ones_col[:], start=True, stop=True)
            c0_sb = consts.tile([1, 1], f32, tag="c0_sb")
            nc.scalar.mul(c0_sb[:], c0_ps[:], 0.5)
            c0b_ps = prep_ps.tile([128, 1], f32, tag="c0b")
            nc.tensor.matmul(c0b_ps[:], ones_row[:], c0_sb[:], start=True, stop=True)
            c0b = consts.tile([128, 1], f32, tag="c0b_sb")
            nc.vector.tensor_copy(c0b[:], c0b_ps[:])

            cu_ps = prep_ps.tile([128, 1], f32, tag="cu")
            nc.tensor.matmul(cu_ps[:], Wu_sb[:], bT[:], start=True, stop=True)
            cu_sb = consts.tile([128, 1], f32, tag="cu_sb")
            nc.vector.tensor_copy(cu_sb[:], cu_ps[:])
            cv_ps = prep_ps.tile([128, 1], f32, tag="cv")
            nc.tensor.matmul(cv_ps[:], Wv_sb[:], bT[:], start=True, stop=True)
            tb_sb = consts.tile([128, 1], f32, tag="tb")
            nc.scalar.add(tb_sb[:], cv_ps[:], bv_col[:])
            nc.vector.tensor_add(tb_sb[:], tb_sb[:], cu_sb[:])

            vT_sb = consts.tile([128, g_core], f32, tag="vT")
            fl_r = flast[:, :].rearrange("(c p) i -> c p i", p=128)
            for c in range(g_core // 128):
                flc = flt_pool.tile([128, IN_DIM], f32)
                nc.sync.dma_start(flc[:], fl_r[c])
                flT_ps = prep_ps.tile([128, 128], f32, tag="flT")
                nc.tensor.transpose(flT_ps[:], flc[:], ident_f[:])
                flT_sb = flt_pool.tile([128, 128], f32, tag="flT_sb")
                nc.vector.tensor_copy(flT_sb[:], flT_ps[:])
                vps = prep_ps.tile([128, 128], f32, tag="vps")
                nc.tensor.matmul(vps[:], Wv_s[:], flT_sb[:], start=True, stop=True)
                nc.scalar.add(vT_sb[:, ts(c, 128)], vps[:], tb_sb[:])

            # vR32[q, w, :] = v for graph 32w+q (fp16 rows on partitions
            # 0-31). The v-fold matmul vR32^T @ ind32 adds v_g into the uT
            # PSUM accumulation so tanh needs no per-graph bias and can span
            # 1024 columns per instruction (the ACT access bubble is ~185ns
            # per instruction — per-graph bias tanh wastes ~47us on it).
            vR32 = consts.tile([32, g_core // 32, 128], f16, tag="vR32")
            for w in range(g_core // 32):
                vtp = prep_ps.tile([128, 128], f32, tag="vps")
                nc.tensor.transpose(vtp[0:32, :], vT_sb[:, ts(w, 32)],
                                    ident_f[:])
                nc.vector.tensor_copy(vR32[:, w, :], vtp[0:32, :])

        # ---------------- Phase B: main pass (fp16) ----------------
        # Pool with UNNORMALIZED exp weights into one device-wide PSUM bank;
        # 1/z and the +b fold are applied after W_out where layout is
        # row-major. Block = 2 pchunks (2 graphs, 512 nodes). exp / Z are
        # batched over pairs of blocks to amortize ACT/PE instruction
        # overhead.
        z2_sb = consts.tile([1, 2 * g_core], f32, tag="z2row")
        with tc.tile_pool(name="ps_pz", bufs=1, space="PSUM") as ps_pz:
          PZ = ps_pz.tile([128, g_core], f32)
          with tc.tile_pool(name="pb_sb", bufs=4) as pb_sb, \
               tc.tile_pool(name="pb_w", bufs=3) as pb_w, \
               tc.tile_pool(name="ps_ft", bufs=2, space="PSUM") as ps_ft, \
               tc.tile_pool(name="ps_u", bufs=2, space="PSUM") as ps_u, \
               tc.tile_pool(name="ps_e", bufs=1, space="PSUM") as ps_e:
            for ng in range(n_grps):
                ftg = pfeat.tile([128, PCH_GRP, 2, 128], f16)
                nc.sync.dma_start(ftg[:], feat_g[ng])
                for quad in range(PCH_GRP // 8):       # 8 pchunks per quad
                    # cols 0-15: e logits; cols 16-31 (row 0): Z partials
                    ez_ps = ps_e.tile([128, 32], f32)
                    wT2 = pb_w.tile([128, 16], f16, tag="wT")
                    for half2 in range(2):
                        # 2-block = 4 pchunks = graphs (g0 .. g0+3)
                        pc0 = quad * 8 + half2 * 4
                        g0 = ng * PCH_GRP + pc0
                        w32 = g0 // 32
                        t32 = (g0 % 32) // 4
                        u2_ps = ps_u.tile([128, 1024], f32)
                        for h in range(2):
                            fT_ps = ps_ft.tile([128, 512], f16)
                            for c in range(4):
                                nc.tensor.transpose(
                                    fT_ps[:, ts(c, 128)],
                                    ftg[:, pc0 + 2 * h + c // 2, c % 2, :],
                                    ident_h[:])
                            fT_sb = pb_sb.tile([128, 512], f16, tag="fT")
                            nc.vector.tensor_copy(fT_sb[:], fT_ps[:])
                            nc.tensor.matmul(
                                u2_ps[:, ts(h, 512)], vR32[:, w32, :],
                                ind32[:, 1024 * t32 + 512 * h:
                                      1024 * t32 + 512 * h + 512],
                                start=True, stop=False)
                            nc.tensor.matmul(u2_ps[:, ts(h, 512)],
                                             Wu_h[:], fT_sb[:],
                                             start=False, stop=True)
                        sig2 = pb_sb.tile([128, 1024], f16, tag="sigT")
                        nc.scalar.activation(sig2[:], u2_ps[:],
                                             AF.Tanh, scale=0.5)
                        for c in range(8):
                            nc.tensor.matmul(
                                ez_ps[:, half2 * 8 + c:half2 * 8 + c + 1],
                                sig2[:, ts(c, 128)], we_h[:],
                                start=True, stop=True)
                    # exp over 16 half-graph weight columns (4 blocks)
                    nc.scalar.activation(wT2[:], ez_ps[:, 0:16], AF.Exp,
                                         bias=c0b[:], scale=1.0)
                    nb16 = 2 * ng + quad
                    mmz = nc.tensor.matmul(ez_ps[0:1, 16:32], ones_h[:],
                                           wT2[:], start=True, stop=True,
                                           skip_group_check=True)
                    mmz.is_weight_onezero = True
                    nc.vector.tensor_copy(z2_sb[:, ts(nb16, 16)],
                                          ez_ps[0:1, 16:32])
                    for c16 in range(16):
                        pc = quad * 8 + c16 // 2
                        g = ng * PCH_GRP + pc
                        nc.tensor.matmul(PZ[:, g:g + 1],
                                         ftg[:, pc, c16 % 2, :],
                                         wT2[:, c16:c16 + 1],
                                         start=(c16 % 2 == 0),
                                         stop=(c16 % 2 == 1),
                                         skip_group_check=True)

          # copy pooled results out of PSUM so those banks free up for the
          # tail
          poolRaw = consts.tile([128, g_core], f32, tag="poolRaw")
          nc.vector.tensor_copy(poolRaw[:], PZ[:])

        # ---------------- Tail: W_out + 1/z + output ----------------
        with tc.tile_pool(name="tail_sb", bufs=2) as tail_sb, \
             tc.tile_pool(name="tail_ps", bufs=1, space="PSUM") as tail_ps:
              # fold Z2 chunk pairs -> zrow [1, g_core]
              z2v = z2_sb[:].rearrange("o (g two) -> o g two", two=2)
              zrow = consts.tile([1, g_core], f32, tag="zrow")
              nc.vector.tensor_add(zrow[:].rearrange("o (g one) -> o g one", one=1),
                                   z2v[:, :, 0:1], z2v[:, :, 1:2])
              rz_row = consts.tile([1, g_core], f32, tag="rz_row")
              nc.vector.reciprocal(rz_row[:], zrow[:])

              # W_out folded with a;  c_out = b @ W_out broadcast to rows
              Wout_a = consts.tile([128, OUT_DIM], f32, tag="Wout_a")
              nc.vector.tensor_scalar_mul(Wout_a[:], Wout_sb[:], aT[:])
              co_ps = tail_ps.tile([128, 2], f32, tag="co")
              for h in range(2):
                  nc.tensor.matmul(co_ps[:, h:h + 1], Wout_sb[:, ts(h, 128)],
                                   bT[:], start=True, stop=True)
              co_sb = consts.tile([128, 2], f32, tag="co_sb")
              nc.vector.tensor_copy(co_sb[:], co_ps[:])
              cor_ps = tail_ps.tile([1, 2, 128], f32, tag="cor")
              for h in range(2):
                  nc.tensor.transpose(cor_ps[:, h, :], co_sb[:, h:h + 1],
                                      ident_f[:])
              co_row = consts.tile([1, 2, 128], f32, tag="co_row")
              nc.vector.tensor_copy(co_row[:], cor_ps[:])
              cob_ps = tail_ps.tile([128, 2, 128], f32, tag="cob")
              nc.tensor.matmul(cob_ps[:], ones_row[:],
                               co_row[:].rearrange("o h d -> o (h d)"),
                               start=True, stop=True)
              co_bc = consts.tile([128, 2, 128], f32, tag="co_bc")
              nc.vector.tensor_copy(co_bc[:], cob_ps[:])

              rstT_sb = []
              for h in range(2):
                  rp = tail_ps.tile([128, g_core], f32, tag="rstT")
                  nc.tensor.matmul(rp[:], Wout_a[:, ts(h, 128)], poolRaw[:],
                                   start=True, stop=True)
                  rs_sb = tail_sb.tile([128, g_core], f32, tag="rstT_sb")
                  nc.vector.tensor_copy(rs_sb[:], rp[:])
                  rstT_sb.append(rs_sb)
              rst_r = rst[:, :].rearrange("(gc p) o -> gc p o", p=128)
              for gc in range(g_core // 128):
                  rzT_ps = tail_ps.tile([128, 1], f32, tag="rzT")
                  nc.tensor.transpose(rzT_ps[:], rz_row[:, ts(gc, 128)],
                                      ident_f[0:1, 0:1])
                  rzT = tail_sb.tile([128, 1], f32, tag="rzT_sb")
                  nc.vector.tensor_copy(rzT[:], rzT_ps[:])
                  rt_ps = tail_ps.tile([128, 2, 128], f32, tag="rt")
                  for h in range(2):
                      nc.tensor.transpose(rt_ps[:, h, :],
                                          rstT_sb[h][:, ts(gc, 128)],
                                          ident_f[:])
                  rt_sb = tail_sb.tile([128, 2, 128], f32, tag="rt_sb")
                  nc.vector.tensor_scalar_mul(rt_sb[:], rt_ps[:], rzT[:])
                  nc.vector.tensor_add(rt_sb[:], rt_sb[:], co_bc[:])
                  nc.sync.dma_start(rst_r[gc],
                                    rt_sb[:].rearrange("p h o -> p (h o)"))


def run_cores(in_maps, n_cores, g_core, trace=False):
    import concourse.bass_utils as bass_utils
    nc = build_nc(n_cores, g_core)
    return bass_utils.run_bass_kernel_spmd(
        nc, in_maps, core_ids=list(range(n_cores)), trace=trace)


def make_in_maps(inputs):
    feat = np.ascontiguousarray(inputs["feat"], np.float32)
    last = np.asarray(inputs["last_nodes"]).astype(np.int64)
    flast_full = np.ascontiguousarray(feat[last])
    feat_h = feat.astype(np.float16)
    in_maps = []
    for d in range(N_CORES):
        in_maps.append({
            "feat": feat_h[d * N_CORE:(d + 1) * N_CORE],
            "flast": flast_full[d * G_CORE:(d + 1) * G_CORE],
            **{k: np.ascontiguousarray(inputs[k], np.float32)
               for k in ("W_u", "W_v", "b_v", "w_e", "W_out", "gamma",
                         "beta")}})
    return in_maps


def _numpy_fallback(feat, gamma, beta, W_u, W_v, b_v, w_e, W_out,
                    segment_ids, last_nodes):
    mean = feat.mean(0)
    var = ((feat - mean) ** 2).mean(0)
    x = (feat - mean) / np.sqrt(var + BN_EPS) * gamma + beta
    fu = x @ W_u
    fv = x[last_nodes] @ W_v + b_v
    e = (1.0 / (1.0 + np.exp(-(fu + fv[segment_ids]))) @ w_e)[:, 0]
    G = int(segment_ids.max()) + 1
    m = np.full(G, -np.inf, np.float32)
    np.maximum.at(m, segment_ids, e)
    ex = np.exp(e - m[segment_ids])
    z = np.zeros(G, np.float32)
    np.add.at(z, segment_ids, ex)
    alpha = ex / z[segment_ids]
    rstv = np.zeros((G, feat.shape[1]), np.float32)
    np.add.at(rstv, segment_ids, x * alpha[:, None])
    return (rstv @ W_out).astype(np.float32)


def kernel(**inputs):
    feat = np.asarray(inputs["feat"])
    seg = np.asarray(inputs["segment_ids"])
    last = np.asarray(inputs["last_nodes"])
    expected_seg = np.repeat(np.arange(NUM_GRAPHS, dtype=np.int64),
                             NODES_PER_GRAPH)
    if feat.shape != (N_TOTAL, IN_DIM) or \
            not np.array_equal(seg.astype(np.int64), expected_seg):
        return _numpy_fallback(
            np.asarray(inputs["feat"], np.float32),
            np.asarray(inputs["gamma"], np.float32),
            np.asarray(inputs["beta"], np.float32),
            np.asarray(inputs["W_u"], np.float32),
            np.asarray(inputs["W_v"], np.float32),
            np.asarray(inputs["b_v"], np.float32),
            np.asarray(inputs["w_e"], np.float32),
            np.asarray(inputs["W_out"], np.float32),
            seg.astype(np.int64), last.astype(np.int64))

    in_maps = make_in_maps(inputs)
    res = run_cores(in_maps, N_CORES, G_CORE)
    out = np.concatenate([res.results[d]["rst"] for d in range(N_CORES)],
                         axis=0)
    return out.astype(np.float32)


# revision 25
# speedup vs baseline: 1.3619x; 1.3619x over previous
# kernel.py — self-contained Trainium2 Bass kernel for nn_AttnReadout.
# Sharding: graph-level data parallel. Device d gets 512 contiguous graphs
# (131072 nodes). BN stats via per-device partial sums + AllReduce.
# sigmoid(y) computed as 0.5 + 0.5*tanh(y/2) so the whole inner loop stays
# on one ACT table set (tanh+exp coexist in exp_and_others).
#
# fp16 data path: feat is shipped and streamed as fp16, which halves both
# host->device transfer and HBM traffic and runs the PE matmul streams at
# 1 cycle/row (fp32 runs at 4). BN statistics, the attention softmax and
# the output tail accumulate in fp32 (PSUM); measured end-to-end relative
# error vs the fp32 reference is ~3e-4.
#
# DMA layout: feat tiles are loaded "pair-interleaved" — partition p holds
# nodes (2p, 2p+1) of a 256-node window — so every DMA descriptor moves a
# 512-byte contiguous run (two 256B feature rows). 256B runs pay a 2x DMA
# latency penalty on TRN2; this layout runs at full HBM bandwidth. The
# within-graph node permutation is harmless: every per-node quantity is
# computed column-consistently and graphs align with 256-node windows.
import os
import sys

sys.path.insert(0, "/opt/trn_rl_repo")
os.environ.setdefault("JAX_PLATFORMS", "axon")

import numpy as np

NUM_GRAPHS = 4096
NODES_PER_GRAPH = 256
N_TOTAL = NUM_GRAPHS * NODES_PER_GRAPH
IN_DIM = 128
HID_DIM = 128
OUT_DIM = 256
BN_EPS = 1e-5
N_CORES = 8

G_CORE = NUM_GRAPHS // N_CORES            # 512 graphs
N_CORE = G_CORE * NODES_PER_GRAPH         # 131072 nodes
CHUNK = 128
PCH_GRP = 16                               # pchunks (=graphs) per DMA group
GRP_NODES = PCH_GRP * NODES_PER_GRAPH      # 4096 nodes / group (1 MiB fp16)

_CACHE = {}


def build_nc(n_cores, g_core):
    import concourse.bass as bass
    import concourse.bacc as bacc
    import concourse.tile as tile
    from concourse import mybir
    from concourse.masks import make_identity

    key = (n_cores, g_core)
    if key in _CACHE:
        return _CACHE[key]

    f32 = mybir.dt.float32
    f16 = mybir.dt.float16
    nc = bacc.Bacc("TRN2", target_bir_lowering=False, debug=False,
                   enable_asserts=False, num_devices=n_cores)
    n_core = g_core * NODES_PER_GRAPH
    feat = nc.dram_tensor("feat", [n_core, IN_DIM], f16, kind="ExternalInput")
    flast = nc.dram_tensor("flast", [g_core, IN_DIM], f32, kind="ExternalInput")
    W_u = nc.dram_tensor("W_u", [IN_DIM, HID_DIM], f32, kind="ExternalInput")
    W_v = nc.dram_tensor("W_v", [IN_DIM, HID_DIM], f32, kind="ExternalInput")
    b_v = nc.dram_tensor("b_v", [HID_DIM], f32, kind="ExternalInput")
    w_e = nc.dram_tensor("w_e", [HID_DIM, 1], f32, kind="ExternalInput")
    W_out = nc.dram_tensor("W_out", [IN_DIM, OUT_DIM], f32, kind="ExternalInput")
    gamma = nc.dram_tensor("gamma", [IN_DIM], f32, kind="ExternalInput")
    beta = nc.dram_tensor("beta", [IN_DIM], f32, kind="ExternalInput")
    rst = nc.dram_tensor("rst", [g_core, OUT_DIM], f32, kind="ExternalOutput")

    with tile.TileContext(nc) as tc:
        _emit(nc, tc, bass, tile, mybir, make_identity,
              feat, flast, W_u, W_v, b_v, w_e, W_out, gamma, beta, rst,
              n_cores, g_core)
    nc.compile()
    _CACHE[key] = nc
    return nc


def _emit(nc, tc, bass, tile, mybir, make_identity,
          feat, flast, W_u, W_v, b_v, w_e, W_out, gamma, beta, rst,
          n_cores, g_core):
    from contextlib import ExitStack

    f32 = mybir.dt.float32
    f16 = mybir.dt.float16
    AF = mybir.ActivationFunctionType
    ts = bass.ts
    n_core = g_core * NODES_PER_GRAPH
    n_total = n_core * n_cores
    n_grps = n_core // GRP_NODES              # 32

    # pair-interleaved group view: group ng, partition p, pchunk c, layer q
    # holds node ng*4096 + c*256 + 2p + q. Innermost (q i) = 512B contiguous.
    feat_g = feat[:, :].rearrange("(ng c p q) i -> ng p c q i",
                                  p=CHUNK, c=PCH_GRP, q=2)

    ctx = ExitStack()
    with ctx:
        consts = ctx.enter_context(tc.tile_pool(name="consts", bufs=1))
        ident_h = consts.tile([128, 128], f16, tag="ident_h")
        make_identity(nc, ident_h[:])
        ident_f = consts.tile([128, 128], f32, tag="ident_f")
        make_identity(nc, ident_f[:])
        ones_h = consts.tile([128, 1], f16, tag="ones_h")
        nc.vector.memset(ones_h[:], 1.0)
        ones_col = consts.tile([128, 1], f32, tag="ones_f")
        nc.vector.memset(ones_col[:], 1.0)
        ones_row = consts.tile([1, 128], f32, tag="ones_r")
        nc.vector.memset(ones_row[:], 1.0)

        # ---------------- Phase A: BN stats (fp16 streams) ----------------
        # One feat pool shared by both phases so phase-B prefetch can start
        # while phase A drains (stack-allocated pools would serialize).
        pfeat = ctx.enter_context(tc.tile_pool(name="pfeat", bufs=3))
        with tc.tile_pool(name="pa_sq", bufs=2) as pa_sq, \
             tc.tile_pool(name="pa_ps", bufs=1, space="PSUM") as pa_ps:
            ps_sum = pa_ps.tile([1, 512], f32, tag="sum")
            ps_sq = pa_ps.tile([1, 512], f32, tag="sq")
            for ng in range(n_grps):
                ft = pfeat.tile([128, PCH_GRP, 2, 128], f16)
                nc.sync.dma_start(ft[:], feat_g[ng])
                sq = pa_sq.tile([128, PCH_GRP, 2, 128], f16)
                nc.vector.tensor_mul(sq[:], ft[:], ft[:])
                for j in range(PCH_GRP // 2):
                    first = (ng == 0 and j == 0)
                    last = (ng == n_grps - 1 and j == PCH_GRP // 2 - 1)
                    sl = slice(2 * j, 2 * j + 2)
                    mm1 = nc.tensor.matmul(ps_sum[:], ones_h[:],
                                           ft[:, sl, :, :],
                                           start=first, stop=last,
                                           skip_group_check=True)
                    mm2 = nc.tensor.matmul(ps_sq[:], ones_h[:],
                                           sq[:, sl, :, :],
                                           start=first, stop=last,
                                           skip_group_check=True)
                    # all-ones stationary: let walrus use the 0/1-weight path
                    mm1.is_weight_onezero = True
                    mm2.is_weight_onezero = True
            stats_sb = consts.tile([1, 1024], f32, tag="stats")
            nc.vector.tensor_copy(stats_sb[:, 0:512], ps_sum[:])
            nc.vector.tensor_copy(stats_sb[:, 512:1024], ps_sq[:])

        # ---------------- AllGather of stats + local fold ----------------
        # AllGather costs ~x1.9 less than AllReduce on TRN2 for tiny
        # payloads; the 8-way sum is 2 cheap f32 matmuls against ones.
        gstats = consts.tile([1, 1024], f32, tag="gstats")
        gst_sb = consts.tile([n_cores, 1024], f32, tag="gst_sb")
        if n_cores > 1:
            with tc.tile_pool(name="dram", bufs=1, space="DRAM") as dram:
                cin = dram.tile([1, 1024], f32, tag="cin")
                cout = dram.tile([n_cores, 1024], f32, tag="cout")
                nc.gpsimd.dma_start(cin[:], stats_sb[:])
                nc.gpsimd.collective_compute(
                    "AllGather", mybir.AluOpType.bypass,
                    replica_groups=[list(range(n_cores))],
                    ins=[cin.opt()], outs=[cout.opt()])
                nc.gpsimd.dma_start(gst_sb[:], cout[:, :])
            with tc.tile_pool(name="ag_ps", bufs=1, space="PSUM") as ag_ps:
                agp = ag_ps.tile([1, 1024], f32, tag="agp")
                for h in range(2):
                    nc.tensor.matmul(agp[:, ts(h, 512)],
                                     ones_col[0:n_cores, :],
                                     gst_sb[:, ts(h, 512)],
                                     start=True, stop=True)
                nc.vector.tensor_copy(gstats[:], agp[:])
        else:
            nc.vector.tensor_copy(gstats[:], stats_sb[:])

        # fold 4 sub-chunk partials -> [1,128]; a = gamma*rsqrt(var+eps),
        # b = beta - mean*a
        srow = consts.tile([1, 128], f32, tag="srow")
        qrow = consts.tile([1, 128], f32, tag="qrow")
        t0 = consts.tile([1, 128], f32, tag="t0")
        t1 = consts.tile([1, 128], f32, tag="t1")
        nc.vector.tensor_add(t0[:], gstats[:, 0:128], gstats[:, 128:256])
        nc.vector.tensor_add(t1[:], gstats[:, 256:384], gstats[:, 384:512])
        nc.vector.tensor_add(srow[:], t0[:], t1[:])
        nc.vector.tensor_add(t0[:], gstats[:, 512:640], gstats[:, 640:768])
        nc.vector.tensor_add(t1[:], gstats[:, 768:896], gstats[:, 896:1024])
        nc.vector.tensor_add(qrow[:], t0[:], t1[:])

        mean_r = consts.tile([1, 128], f32, tag="mean")
        ex2_r = consts.tile([1, 128], f32, tag="ex2")
        nc.scalar.mul(mean_r[:], srow[:], 1.0 / n_total)
        nc.scalar.mul(ex2_r[:], qrow[:], 1.0 / n_total)
        var_r = consts.tile([1, 128], f32, tag="var")
        nc.vector.tensor_mul(t0[:], mean_r[:], mean_r[:])
        nc.vector.tensor_scalar_mul(t0[:], t0[:], -1.0)
        nc.vector.tensor_add(var_r[:], t0[:], ex2_r[:])
        eps_t = consts.tile([1, 1], f32, tag="eps")
        nc.vector.memset(eps_t[:], BN_EPS)
        sd_r = consts.tile([1, 128], f32, tag="sd")
        nc.scalar.activation(sd_r[:], var_r[:], AF.Sqrt, bias=eps_t[:], scale=1.0)
        rs_r = consts.tile([1, 128], f32, tag="rs")
        nc.vector.reciprocal(rs_r[:], sd_r[:])

        grow = consts.tile([1, 128], f32, tag="grow")
        brow = consts.tile([1, 128], f32, tag="brow")
        nc.sync.dma_start(grow[:], gamma[:].rearrange("(o p) -> o p", o=1))
        nc.sync.dma_start(brow[:], beta[:].rearrange("(o p) -> o p", o=1))
        a_r = consts.tile([1, 128], f32, tag="a_r")
        b_r = consts.tile([1, 128], f32, tag="b_r")
        nc.vector.tensor_mul(a_r[:], rs_r[:], grow[:])
        nc.vector.tensor_mul(t0[:], mean_r[:], a_r[:])
        nc.vector.tensor_scalar_mul(t0[:], t0[:], -1.0)
        nc.vector.tensor_add(b_r[:], t0[:], brow[:])

        # folded weights + per-graph bias matrix vT (scaled by 0.5 for tanh)
        with tc.tile_pool(name="prep_ps", bufs=1, space="PSUM") as prep_ps, \
             tc.tile_pool(name="flt", bufs=2) as flt_pool:
            aT = consts.tile([128, 1], f32, tag="aT")
            bT = consts.tile([128, 1], f32, tag="bT")
            pT = prep_ps.tile([128, 1], f32, tag="pT")
            nc.tensor.transpose(pT[:], a_r[:], ident_f[0:1, 0:1])
            nc.vector.tensor_copy(aT[:], pT[:])
            pT2 = prep_ps.tile([128, 1], f32, tag="pT2")
            nc.tensor.transpose(pT2[:], b_r[:], ident_f[0:1, 0:1])
            nc.vector.tensor_copy(bT[:], pT2[:])

            Wu_sb = consts.tile([128, HID_DIM], f32, tag="Wu")
            Wv_sb = consts.tile([128, HID_DIM], f32, tag="Wv")
            Wout_sb = consts.tile([128, OUT_DIM], f32, tag="Wout")
            we_sb = consts.tile([128, 1], f32, tag="we")
            bv_col = consts.tile([128, 1], f32, tag="bv")
            nc.sync.dma_start(Wu_sb[:], W_u[:, :])
            nc.sync.dma_start(Wv_sb[:], W_v[:, :])
            nc.sync.dma_start(Wout_sb[:], W_out[:, :])
            nc.sync.dma_start(we_sb[:], w_e[:, :])
            nc.sync.dma_start(bv_col[:], b_v[:].rearrange("(p o) -> p o", o=1))

            Wu_s = consts.tile([128, HID_DIM], f32, tag="Wu_s")
            Wv_s = consts.tile([128, HID_DIM], f32, tag="Wv_s")
            nc.vector.tensor_scalar_mul(Wu_s[:], Wu_sb[:], aT[:])
            nc.vector.tensor_scalar_mul(Wv_s[:], Wv_sb[:], aT[:])
            # fp16 copy for the hot loop
            Wu_h = consts.tile([128, HID_DIM], f16, tag="Wu_h")
            nc.vector.tensor_copy(Wu_h[:], Wu_s[:])

            # we_h = 0.5*w_e (fp16) ; c0b = 0.5*sum(w_e) broadcast column
            we_h = consts.tile([128, 1], f16, tag="we_h")
            nc.scalar.mul(we_h[:], we_sb[:], 0.5)
            c0_ps = prep_ps.tile([1, 1], f32, tag="c0")
            nc.tensor.matmul(c0_ps[:], we_sb[:], ones_col[:], start=True, stop=True)
            c0_sb = consts.tile([1, 1], f32, tag="c0_sb")
            nc.scalar.mul(c0_sb[:], c0_ps[:], 0.5)
            c0b_ps = prep_ps.tile([128, 1], f32, tag="c0b")
            nc.tensor.matmul(c0b_ps[:], ones_row[:], c0_sb[:], start=True, stop=True)
            c0b = consts.tile([128, 1], f32, tag="c0b_sb")
            nc.vector.tensor_copy(c0b[:], c0b_ps[:])

            cu_ps = prep_ps.tile([128, 1], f32, tag="cu")
            nc.tensor.matmul(cu_ps[:], Wu_sb[:], bT[:], start=True, stop=True)
            cu_sb = consts.tile([128, 1], f32, tag="cu_sb")
            nc.vector.tensor_copy(cu_sb[:], cu_ps[:])
            cv_ps = prep_ps.tile([128, 1], f32, tag="cv")
            nc.tensor.matmul(cv_ps[:], Wv_sb[:], bT[:], start=True, stop=True)
            tb_sb = consts.tile([128, 1], f32, tag="tb")
            nc.scalar.add(tb_sb[:], cv_ps[:], bv_col[:])
            nc.vector.tensor_add(tb_sb[:], tb_sb[:], cu_sb[:])

            vT_sb = consts.tile([128, g_core], f32, tag="vT")
            fl_r = flast[:, :].rearrange("(c p) i -> c p i", p=128)
            for c in range(g_core // 128):
                flc = flt_pool.tile([128, IN_DIM], f32)
                nc.sync.dma_start(flc[:], fl_r[c])
                flT_ps = prep_ps.tile([128, 128], f32, tag="flT")
                nc.tensor.transpose(flT_ps[:], flc[:], ident_f[:])
                flT_sb = flt_pool.tile([128, 128], f32, tag="flT_sb")
                nc.vector.tensor_copy(flT_sb[:], flT_ps[:])
                vps = prep_ps.tile([128, 128], f32, tag="vps")
                nc.tensor.matmul(vps[:], Wv_s[:], flT_sb[:], start=True, stop=True)
                nc.scalar.add(vT_sb[:, ts(c, 128)], vps[:], tb_sb[:])

            # scale by 0.5 for the tanh form of sigmoid
            nc.vector.tensor_scalar_mul(vT_sb[:], vT_sb[:], 0.5)

        # ---------------- Phase B: main pass (fp16) ----------------
        # Pool with UNNORMALIZED exp weights into one device-wide PSUM bank;
        # 1/z and the +b fold are applied after W_out where layout is
        # row-major. Block = 2 pchunks (2 graphs, 512 nodes). exp / Z are
        # batched over pairs of blocks to amortize ACT/PE instruction
        # overhead.
        z2_sb = consts.tile([1, 2 * g_core], f32, tag="z2row")
        with tc.tile_pool(name="ps_pz", bufs=1, space="PSUM") as ps_pz:
          PZ = ps_pz.tile([128, g_core], f32)
          with tc.tile_pool(name="pb_sb", bufs=4) as pb_sb, \
               tc.tile_pool(name="pb_w", bufs=3) as pb_w, \
               tc.tile_pool(name="ps_ft", bufs=2, space="PSUM") as ps_ft, \
               tc.tile_pool(name="ps_u", bufs=3, space="PSUM") as ps_u, \
               tc.tile_pool(name="ps_e", bufs=2, space="PSUM") as ps_e:
            for ng in range(n_grps):
                ftg = pfeat.tile([128, PCH_GRP, 2, 128], f16)
                nc.sync.dma_start(ftg[:], feat_g[ng])
                for quad in range(PCH_GRP // 8):       # 8 pchunks per quad
                    # cols 0-15: e logits; cols 16-31 (row 0): Z partials
                    ez_ps = ps_e.tile([128, 32], f32)
                    wT2 = pb_w.tile([128, 16], f16, tag="wT")
                    for blk in range(4):
                        # block = 2 pchunks = graphs (g0, g0+1)
                        pc0 = quad * 8 + blk * 2
                        g0 = ng * PCH_GRP + pc0
                        fT_ps = ps_ft.tile([128, 512], f16)
                        for c in range(4):
                            nc.tensor.transpose(
                                fT_ps[:, ts(c, 128)],
                                ftg[:, pc0 + c // 2, c % 2, :],
                                ident_h[:])
                        fT_sb = pb_sb.tile([128, 512], f16, tag="fT")
                        nc.vector.tensor_copy(fT_sb[:], fT_ps[:])
                        uT_ps = ps_u.tile([128, 512], f32)
                        nc.tensor.matmul(uT_ps[:], Wu_h[:], fT_sb[:],
                                         start=True, stop=True)
                        sigT = pb_sb.tile([128, 512], f16, tag="sigT")
                        for gb in range(2):
                            nc.scalar.activation(
                                sigT[:, ts(gb, 256)],
                                uT_ps[:, ts(gb, 256)],
                                AF.Tanh, bias=vT_sb[:, g0 + gb:g0 + gb + 1],
                                scale=0.5)
                        for c in range(4):
                            nc.tensor.matmul(
                                ez_ps[:, blk * 4 + c:blk * 4 + c + 1],
                                sigT[:, ts(c, 128)], we_h[:],
                                start=True, stop=True)
                    # exp over 16 half-graph weight columns (4 blocks)
                    nc.scalar.activation(wT2[:], ez_ps[:, 0:16], AF.Exp,
                                         bias=c0b[:], scale=1.0)
                    nb16 = 2 * ng + quad
                    mmz = nc.tensor.matmul(ez_ps[0:1, 16:32], ones_h[:],
                                           wT2[:], start=True, stop=True,
                                           skip_group_check=True)
                    mmz.is_weight_onezero = True
                    nc.vector.tensor_copy(z2_sb[:, ts(nb16, 16)],
                                          ez_ps[0:1, 16:32])
                    for c16 in range(16):
                        pc = quad * 8 + c16 // 2
                        g = ng * PCH_GRP + pc
                        nc.tensor.matmul(PZ[:, g:g + 1],
                                         ftg[:, pc, c16 % 2, :],
                                         wT2[:, c16:c16 + 1],
                                         start=(c16 % 2 == 0),
                                         stop=(c16 % 2 == 1),
                                         skip_group_check=True)

          # copy pooled results out of PSUM so those banks free up for the
          # tail
          poolRaw = consts.tile([128, g_core], f32, tag="poolRaw")
          nc.vector.tensor_copy(poolRaw[:], PZ[:])

        # ---------------- Tail: W_out + 1/z + output ----------------
        with tc.tile_pool(name="tail_sb", bufs=2) as tail_sb, \
             tc.tile_pool(name="tail_ps", bufs=1, space="PSUM") as tail_ps:
              # fold Z2 chunk pairs -> zrow [1, g_core]
              z2v = z2_sb[:].rearrange("o (g two) -> o g two", two=2)
              zrow = consts.tile([1, g_core], f32, tag="zrow")
              nc.vector.tensor_add(zrow[:].rearrange("o (g one) -> o g one", one=1),
                                   z2v[:, :, 0:1], z2v[:, :, 1:2])
              rz_row = consts.tile([1, g_core], f32, tag="rz_row")
              nc.vector.reciprocal(rz_row[:], zrow[:])

              # W_out folded with a;  c_out = b @ W_out broadcast to rows
              Wout_a = consts.tile([128, OUT_DIM], f32, tag="Wout_a")
              nc.vector.tensor_scalar_mul(Wout_a[:], Wout_sb[:], aT[:])
              co_ps = tail_ps.tile([128, 2], f32, tag="co")
              for h in range(2):
                  nc.tensor.matmul(co_ps[:, h:h + 1], Wout_sb[:, ts(h, 128)],
                                   bT[:], start=True, stop=True)
              co_sb = consts.tile([128, 2], f32, tag="co_sb")
              nc.vector.tensor_copy(co_sb[:], co_ps[:])
              cor_ps = tail_ps.tile([1, 2, 128], f32, tag="cor")
              for h in range(2):
                  nc.tensor.transpose(cor_ps[:, h, :], co_sb[:, h:h + 1],
                                      ident_f[:])
              co_row = consts.tile([1, 2, 128], f32, tag="co_row")
              nc.vector.tensor_copy(co_row[:], cor_ps[:])
              cob_ps = tail_ps.tile([128, 2, 128], f32, tag="cob")
              nc.tensor.matmul(cob_ps[:], ones_row[:],
                               co_row[:].rearrange("o h d -> o (h d)"),
                               start=True, stop=True)
              co_bc = consts.tile([128, 2, 128], f32, tag="co_bc")
              nc.vector.tensor_copy(co_bc[:], cob_ps[:])

              rstT_sb = []
              for h in range(2):
                  rp = tail_ps.tile([128, g_core], f32, tag="rstT")
                  nc.tensor.matmul(rp[:], Wout_a[:, ts(h, 128)], poolRaw[:],
                                   start=True, stop=True)
                  rs_sb = tail_sb.tile([128, g_core], f32, tag="rstT_sb")
                  nc.vector.tensor_copy(rs_sb[:], rp[:])
                  rstT_sb.append(rs_sb)
              rst_r = rst[:, :].rearrange("(gc p) o -> gc p o", p=128)
              for gc in range(g_core // 128):
                  rzT_ps = tail_ps.tile([128, 1], f32, tag="rzT")
                  nc.tensor.transpose(rzT_ps[:], rz_row[:, ts(gc, 128)],
                                      ident_f[0:1, 0:1])
                  rzT = tail_sb.tile([128, 1], f32, tag="rzT_sb")
                  nc.vector.tensor_copy(rzT[:], rzT_ps[:])
                  rt_ps = tail_ps.tile([128, 2, 128], f32, tag="rt")
                  for h in range(2):
                      nc.tensor.transpose(rt_ps[:, h, :],
                                          rstT_sb[h][:, ts(gc, 128)],
                                          ident_f[:])
                  rt_sb = tail_sb.tile([128, 2, 128], f32, tag="rt_sb")
                  nc.vector.tensor_scalar_mul(rt_sb[:], rt_ps[:], rzT[:])
                  nc.vector.tensor_add(rt_sb[:], rt_sb[:], co_bc[:])
                  nc.sync.dma_start(rst_r[gc],
                                    rt_sb[:].rearrange("p h o -> p (h o)"))


def run_cores(in_maps, n_cores, g_core, trace=False):
    import concourse.bass_utils as bass_utils
    nc = build_nc(n_cores, g_core)
    return bass_utils.run_bass_kernel_spmd(
        nc, in_maps, core_ids=list(range(n_cores)), trace=trace)


def make_in_maps(inputs):
    feat = np.ascontiguousarray(inputs["feat"], np.float32)
    last = np.asarray(inputs["last_nodes"]).astype(np.int64)
    flast_full = np.ascontiguousarray(feat[last])
    feat_h = feat.astype(np.float16)
    in_maps = []
    for d in range(N_CORES):
        in_maps.append({
            "feat": feat_h[d * N_CORE:(d + 1) * N_CORE],
            "flast": flast_full[d * G_CORE:(d + 1) * G_CORE],
            **{k: np.ascontiguousarray(inputs[k], np.float32)
               for k in ("W_u", "W_v", "b_v", "w_e", "W_out", "gamma",
                         "beta")}})
    return in_maps


def _numpy_fallback(feat, gamma, beta, W_u, W_v, b_v, w_e, W_out,
                    segment_ids, last_nodes):
    mean = feat.mean(0)
    var = ((feat - mean) ** 2).mean(0)
    x = (feat - mean) / np.sqrt(var + BN_EPS) * gamma + beta
    fu = x @ W_u
    fv = x[last_nodes] @ W_v + b_v
    e = (1.0 / (1.0 + np.exp(-(fu + fv[segment_ids]))) @ w_e)[:, 0]
    G = int(segment_ids.max()) + 1
    m = np.full(G, -np.inf, np.float32)
    np.maximum.at(m, segment_ids, e)
    ex = np.exp(e - m[segment_ids])
    z = np.zeros(G, np.float32)
    np.add.at(z, segment_ids, ex)
    alpha = ex / z[segment_ids]
    rstv = np.zeros((G, feat.shape[1]), np.float32)
    np.add.at(rstv, segment_ids, x * alpha[:, None])
    return (rstv @ W_out).astype(np.float32)


def kernel(**inputs):
    feat = np.asarray(inputs["feat"])
    seg = np.asarray(inputs["segment_ids"])
    last = np.asarray(inputs["last_nodes"])
    expected_seg = np.repeat(np.arange(NUM_GRAPHS, dtype=np.int64),
                             NODES_PER_GRAPH)
    if feat.shape != (N_TOTAL, IN_DIM) or \
            not np.array_equal(seg.astype(np.int64), expected_seg):
        return _numpy_fallback(
            np.asarray(inputs["feat"], np.float32),
            np.asarray(inputs["gamma"], np.float32),
            np.asarray(inputs["beta"], np.float32),
            np.asarray(inputs["W_u"], np.float32),
            np.asarray(inputs["W_v"], np.float32),
            np.asarray(inputs["b_v"], np.float32),
            np.asarray(inputs["w_e"], np.float32),
            np.asarray(inputs["W_out"], np.float32),
            seg.astype(np.int64), last.astype(np.int64))

    in_maps = make_in_maps(inputs)
    res = run_cores(in_maps, N_CORES, G_CORE)
    out = np.concatenate([res.results[d]["rst"] for d in range(N_CORES)],
                         axis=0)
    return out.astype(np.float32)


# revision 26
# speedup vs baseline: 1.5325x; 1.1253x over previous
# kernel.py — self-contained Trainium2 Bass kernel for nn_AttnReadout.
# Sharding: graph-level data parallel. Device d gets 512 contiguous graphs
# (131072 nodes). BN stats via per-device partial sums + AllReduce.
# sigmoid(y) computed as 0.5 + 0.5*tanh(y/2) so the whole inner loop stays
# on one ACT table set (tanh+exp coexist in exp_and_others).
#
# fp16 data path: feat is shipped and streamed as fp16, which halves both
# host->device transfer and HBM traffic and runs the PE matmul streams at
# 1 cycle/row (fp32 runs at 4). BN statistics, the attention softmax and
# the output tail accumulate in fp32 (PSUM); measured end-to-end relative
# error vs the fp32 reference is ~3e-4.
#
# DMA layout: feat tiles are loaded "pair-interleaved" — partition p holds
# nodes (2p, 2p+1) of a 256-node window — so every DMA descriptor moves a
# 512-byte contiguous run (two 256B feature rows). 256B runs pay a 2x DMA
# latency penalty on TRN2; this layout runs at full HBM bandwidth. The
# within-graph node permutation is harmless: every per-node quantity is
# computed column-consistently and graphs align with 256-node windows.
import os
import sys

sys.path.insert(0, "/opt/trn_rl_repo")
os.environ.setdefault("JAX_PLATFORMS", "axon")

import numpy as np

NUM_GRAPHS = 4096
NODES_PER_GRAPH = 256
N_TOTAL = NUM_GRAPHS * NODES_PER_GRAPH
IN_DIM = 128
HID_DIM = 128
OUT_DIM = 256
BN_EPS = 1e-5
N_CORES = 8

G_CORE = NUM_GRAPHS // N_CORES            # 512 graphs
N_CORE = G_CORE * NODES_PER_GRAPH         # 131072 nodes
CHUNK = 128
PCH_GRP = 16                               # pchunks (=graphs) per DMA group
GRP_NODES = PCH_GRP * NODES_PER_GRAPH      # 4096 nodes / group (1 MiB fp16)

_CACHE = {}


def build_nc(n_cores, g_core):
    import concourse.bass as bass
    import concourse.bacc as bacc
    import concourse.tile as tile
    from concourse import mybir
    from concourse.masks import make_identity

    key = (n_cores, g_core)
    if key in _CACHE:
        return _CACHE[key]

    f32 = mybir.dt.float32
    f16 = mybir.dt.float16
    nc = bacc.Bacc("TRN2", target_bir_lowering=False, debug=False,
                   enable_asserts=False, num_devices=n_cores)
    n_core = g_core * NODES_PER_GRAPH
    feat = nc.dram_tensor("feat", [n_core, IN_DIM], f16, kind="ExternalInput")
    flast = nc.dram_tensor("flast", [g_core, IN_DIM], f32, kind="ExternalInput")
    W_u = nc.dram_tensor("W_u", [IN_DIM, HID_DIM], f32, kind="ExternalInput")
    W_v = nc.dram_tensor("W_v", [IN_DIM, HID_DIM], f32, kind="ExternalInput")
    b_v = nc.dram_tensor("b_v", [HID_DIM], f32, kind="ExternalInput")
    w_e = nc.dram_tensor("w_e", [HID_DIM, 1], f32, kind="ExternalInput")
    W_out = nc.dram_tensor("W_out", [IN_DIM, OUT_DIM], f32, kind="ExternalInput")
    gamma = nc.dram_tensor("gamma", [IN_DIM], f32, kind="ExternalInput")
    beta = nc.dram_tensor("beta", [IN_DIM], f32, kind="ExternalInput")
    rst = nc.dram_tensor("rst", [g_core, OUT_DIM], f32, kind="ExternalOutput")

    with tile.TileContext(nc) as tc:
        _emit(nc, tc, bass, tile, mybir, make_identity,
              feat, flast, W_u, W_v, b_v, w_e, W_out, gamma, beta, rst,
              n_cores, g_core)
    nc.compile()
    _CACHE[key] = nc
    return nc


def _emit(nc, tc, bass, tile, mybir, make_identity,
          feat, flast, W_u, W_v, b_v, w_e, W_out, gamma, beta, rst,
          n_cores, g_core):
    from contextlib import ExitStack

    f32 = mybir.dt.float32
    f16 = mybir.dt.float16
    AF = mybir.ActivationFunctionType
    ts = bass.ts
    n_core = g_core * NODES_PER_GRAPH
    n_total = n_core * n_cores
    n_grps = n_core // GRP_NODES              # 32

    # pair-interleaved group view: group ng, partition p, pchunk c, layer q
    # holds node ng*4096 + c*256 + 2p + q. Innermost (q i) = 512B contiguous.
    feat_g = feat[:, :].rearrange("(ng c p q) i -> ng p c q i",
                                  p=CHUNK, c=PCH_GRP, q=2)

    ctx = ExitStack()
    with ctx:
        consts = ctx.enter_context(tc.tile_pool(name="consts", bufs=1))
        ident_h = consts.tile([128, 128], f16, tag="ident_h")
        make_identity(nc, ident_h[:])
        ident_f = consts.tile([128, 128], f32, tag="ident_f")
        make_identity(nc, ident_f[:])
        ones_h = consts.tile([128, 1], f16, tag="ones_h")
        nc.vector.memset(ones_h[:], 1.0)
        ones_col = consts.tile([128, 1], f32, tag="ones_f")
        nc.vector.memset(ones_col[:], 1.0)
        ones_row = consts.tile([1, 128], f32, tag="ones_r")
        nc.vector.memset(ones_row[:], 1.0)

        # ---------------- Phase A: BN stats (fp16 streams) ----------------
        # One feat pool shared by both phases so phase-B prefetch can start
        # while phase A drains (stack-allocated pools would serialize).
        pfeat = ctx.enter_context(tc.tile_pool(name="pfeat", bufs=3))
        with tc.tile_pool(name="pa_sq", bufs=2) as pa_sq, \
             tc.tile_pool(name="pa_ps", bufs=1, space="PSUM") as pa_ps:
            ps_sum = pa_ps.tile([1, 512], f32, tag="sum")
            ps_sq = pa_ps.tile([1, 512], f32, tag="sq")
            for ng in range(n_grps):
                ft = pfeat.tile([128, PCH_GRP, 2, 128], f16)
                nc.sync.dma_start(ft[:], feat_g[ng])
                sq = pa_sq.tile([128, PCH_GRP, 2, 128], f16)
                nc.vector.tensor_mul(sq[:, 0:PCH_GRP // 2], ft[:, 0:PCH_GRP // 2],
                                     ft[:, 0:PCH_GRP // 2])
                nc.vector.tensor_mul(sq[:, PCH_GRP // 2:], ft[:, PCH_GRP // 2:],
                                     ft[:, PCH_GRP // 2:])
                for j in range(PCH_GRP // 2):
                    first = (ng == 0 and j == 0)
                    last = (ng == n_grps - 1 and j == PCH_GRP // 2 - 1)
                    sl = slice(2 * j, 2 * j + 2)
                    mm1 = nc.tensor.matmul(ps_sum[:], ones_h[:],
                                           ft[:, sl, :, :],
                                           start=first, stop=last,
                                           skip_group_check=True)
                    mm2 = nc.tensor.matmul(ps_sq[:], ones_h[:],
                                           sq[:, sl, :, :],
                                           start=first, stop=last,
                                           skip_group_check=True)
                    # all-ones stationary: let walrus use the 0/1-weight path
                    mm1.is_weight_onezero = True
                    mm2.is_weight_onezero = True
            stats_sb = consts.tile([1, 1024], f32, tag="stats")
            nc.vector.tensor_copy(stats_sb[:, 0:512], ps_sum[:])
            nc.vector.tensor_copy(stats_sb[:, 512:1024], ps_sq[:])

        # ---------------- AllGather of stats + local fold ----------------
        # AllGather costs ~x1.9 less than AllReduce on TRN2 for tiny
        # payloads; the 8-way sum is 2 cheap f32 matmuls against ones.
        gstats = consts.tile([1, 1024], f32, tag="gstats")
        gst_sb = consts.tile([n_cores, 1024], f32, tag="gst_sb")
        if n_cores > 1:
            with tc.tile_pool(name="dram", bufs=1, space="DRAM") as dram:
                cin = dram.tile([1, 1024], f32, tag="cin")
                cout = dram.tile([n_cores, 1024], f32, tag="cout")
                nc.gpsimd.dma_start(cin[:], stats_sb[:])
                nc.gpsimd.collective_compute(
                    "AllGather", mybir.AluOpType.bypass,
                    replica_groups=[list(range(n_cores))],
                    ins=[cin.opt()], outs=[cout.opt()])
                nc.gpsimd.dma_start(gst_sb[:], cout[:, :])
            with tc.tile_pool(name="ag_ps", bufs=1, space="PSUM") as ag_ps:
                agp = ag_ps.tile([1, 1024], f32, tag="agp")
                for h in range(2):
                    nc.tensor.matmul(agp[:, ts(h, 512)],
                                     ones_col[0:n_cores, :],
                                     gst_sb[:, ts(h, 512)],
                                     start=True, stop=True)
                nc.vector.tensor_copy(gstats[:], agp[:])
        else:
            nc.vector.tensor_copy(gstats[:], stats_sb[:])

        # fold 4 sub-chunk partials -> [1,128]; a = gamma*rsqrt(var+eps),
        # b = beta - mean*a
        srow = consts.tile([1, 128], f32, tag="srow")
        qrow = consts.tile([1, 128], f32, tag="qrow")
        t0 = consts.tile([1, 128], f32, tag="t0")
        t1 = consts.tile([1, 128], f32, tag="t1")
        nc.vector.tensor_add(t0[:], gstats[:, 0:128], gstats[:, 128:256])
        nc.vector.tensor_add(t1[:], gstats[:, 256:384], gstats[:, 384:512])
        nc.vector.tensor_add(srow[:], t0[:], t1[:])
        nc.vector.tensor_add(t0[:], gstats[:, 512:640], gstats[:, 640:768])
        nc.vector.tensor_add(t1[:], gstats[:, 768:896], gstats[:, 896:1024])
        nc.vector.tensor_add(qrow[:], t0[:], t1[:])

        mean_r = consts.tile([1, 128], f32, tag="mean")
        ex2_r = consts.tile([1, 128], f32, tag="ex2")
        nc.scalar.mul(mean_r[:], srow[:], 1.0 / n_total)
        nc.scalar.mul(ex2_r[:], qrow[:], 1.0 / n_total)
        var_r = consts.tile([1, 128], f32, tag="var")
        nc.vector.tensor_mul(t0[:], mean_r[:], mean_r[:])
        nc.vector.tensor_scalar_mul(t0[:], t0[:], -1.0)
        nc.vector.tensor_add(var_r[:], t0[:], ex2_r[:])
        eps_t = consts.tile([1, 1], f32, tag="eps")
        nc.vector.memset(eps_t[:], BN_EPS)
        sd_r = consts.tile([1, 128], f32, tag="sd")
        nc.scalar.activation(sd_r[:], var_r[:], AF.Sqrt, bias=eps_t[:], scale=1.0)
        rs_r = consts.tile([1, 128], f32, tag="rs")
        nc.vector.reciprocal(rs_r[:], sd_r[:])

        grow = consts.tile([1, 128], f32, tag="grow")
        brow = consts.tile([1, 128], f32, tag="brow")
        nc.sync.dma_start(grow[:], gamma[:].rearrange("(o p) -> o p", o=1))
        nc.sync.dma_start(brow[:], beta[:].rearrange("(o p) -> o p", o=1))
        a_r = consts.tile([1, 128], f32, tag="a_r")
        b_r = consts.tile([1, 128], f32, tag="b_r")
        nc.vector.tensor_mul(a_r[:], rs_r[:], grow[:])
        nc.vector.tensor_mul(t0[:], mean_r[:], a_r[:])
        nc.vector.tensor_scalar_mul(t0[:], t0[:], -1.0)
        nc.vector.tensor_add(b_r[:], t0[:], brow[:])

        # folded weights + per-graph bias matrix vT (scaled by 0.5 for tanh)
        with tc.tile_pool(name="prep_ps", bufs=1, space="PSUM") as prep_ps, \
             tc.tile_pool(name="flt", bufs=2) as flt_pool:
            aT = consts.tile([128, 1], f32, tag="aT")
            bT = consts.tile([128, 1], f32, tag="bT")
            pT = prep_ps.tile([128, 1], f32, tag="pT")
            nc.tensor.transpose(pT[:], a_r[:], ident_f[0:1, 0:1])
            nc.vector.tensor_copy(aT[:], pT[:])
            pT2 = prep_ps.tile([128, 1], f32, tag="pT2")
            nc.tensor.transpose(pT2[:], b_r[:], ident_f[0:1, 0:1])
            nc.vector.tensor_copy(bT[:], pT2[:])

            Wu_sb = consts.tile([128, HID_DIM], f32, tag="Wu")
            Wv_sb = consts.tile([128, HID_DIM], f32, tag="Wv")
            Wout_sb = consts.tile([128, OUT_DIM], f32, tag="Wout")
            we_sb = consts.tile([128, 1], f32, tag="we")
            bv_col = consts.tile([128, 1], f32, tag="bv")
            nc.sync.dma_start(Wu_sb[:], W_u[:, :])
            nc.sync.dma_start(Wv_sb[:], W_v[:, :])
            nc.sync.dma_start(Wout_sb[:], W_out[:, :])
            nc.sync.dma_start(we_sb[:], w_e[:, :])
            nc.sync.dma_start(bv_col[:], b_v[:].rearrange("(p o) -> p o", o=1))

            Wu_s = consts.tile([128, HID_DIM], f32, tag="Wu_s")
            Wv_s = consts.tile([128, HID_DIM], f32, tag="Wv_s")
            nc.vector.tensor_scalar_mul(Wu_s[:], Wu_sb[:], aT[:])
            nc.vector.tensor_scalar_mul(Wv_s[:], Wv_sb[:], aT[:])
            # fp16 copy for the hot loop
            Wu_h = consts.tile([128, HID_DIM], f16, tag="Wu_h")
            nc.vector.tensor_copy(Wu_h[:], Wu_s[:])

            # we_h = 0.5*w_e (fp16) ; c0b = 0.5*sum(w_e) broadcast column
            we_h = consts.tile([128, 1], f16, tag="we_h")
            nc.scalar.mul(we_h[:], we_sb[:], 0.5)
            c0_ps = prep_ps.tile([1, 1], f32, tag="c0")
            nc.tensor.matmul(c0_ps[:], we_sb[:], ones_col[:], start=True, stop=True)
            c0_sb = consts.tile([1, 1], f32, tag="c0_sb")
            nc.scalar.mul(c0_sb[:], c0_ps[:], 0.5)
            c0b_ps = prep_ps.tile([128, 1], f32, tag="c0b")
            nc.tensor.matmul(c0b_ps[:], ones_row[:], c0_sb[:], start=True, stop=True)
            c0b = consts.tile([128, 1], f32, tag="c0b_sb")
            nc.vector.tensor_copy(c0b[:], c0b_ps[:])

            cu_ps = prep_ps.tile([128, 1], f32, tag="cu")
            nc.tensor.matmul(cu_ps[:], Wu_sb[:], bT[:], start=True, stop=True)
            cu_sb = consts.tile([128, 1], f32, tag="cu_sb")
            nc.vector.tensor_copy(cu_sb[:], cu_ps[:])
            cv_ps = prep_ps.tile([128, 1], f32, tag="cv")
            nc.tensor.matmul(cv_ps[:], Wv_sb[:], bT[:], start=True, stop=True)
            tb_sb = consts.tile([128, 1], f32, tag="tb")
            nc.scalar.add(tb_sb[:], cv_ps[:], bv_col[:])
            nc.vector.tensor_add(tb_sb[:], tb_sb[:], cu_sb[:])

            vT_sb = consts.tile([128, g_core], f32, tag="vT")
            fl_r = flast[:, :].rearrange("(c p) i -> c p i", p=128)
            for c in range(g_core // 128):
                flc = flt_pool.tile([128, IN_DIM], f32)
                nc.sync.dma_start(flc[:], fl_r[c])
                flT_ps = prep_ps.tile([128, 128], f32, tag="flT")
                nc.tensor.transpose(flT_ps[:], flc[:], ident_f[:])
                flT_sb = flt_pool.tile([128, 128], f32, tag="flT_sb")
                nc.vector.tensor_copy(flT_sb[:], flT_ps[:])
                vps = prep_ps.tile([128, 128], f32, tag="vps")
                nc.tensor.matmul(vps[:], Wv_s[:], flT_sb[:], start=True, stop=True)
                nc.scalar.add(vT_sb[:, ts(c, 128)], vps[:], tb_sb[:])

            # scale by 0.5 for the tanh form of sigmoid
            nc.vector.tensor_scalar_mul(vT_sb[:], vT_sb[:], 0.5)

        # ---------------- Phase B: main pass (fp16) ----------------
        # Pool with UNNORMALIZED exp weights into one device-wide PSUM bank;
        # 1/z and the +b fold are applied after W_out where layout is
        # row-major. Block = 2 pchunks (2 graphs, 512 nodes). exp / Z are
        # batched over pairs of blocks to amortize ACT/PE instruction
        # overhead.
        z2_sb = consts.tile([1, 2 * g_core], f32, tag="z2row")
        with tc.tile_pool(name="ps_pz", bufs=1, space="PSUM") as ps_pz:
          PZ = ps_pz.tile([128, g_core], f32)
          with tc.tile_pool(name="pb_sb", bufs=4) as pb_sb, \
               tc.tile_pool(name="pb_w", bufs=3) as pb_w, \
               tc.tile_pool(name="ps_ft", bufs=2, space="PSUM") as ps_ft, \
               tc.tile_pool(name="ps_u", bufs=3, space="PSUM") as ps_u, \
               tc.tile_pool(name="ps_e", bufs=2, space="PSUM") as ps_e:
            for ng in range(n_grps):
                ftg = pfeat.tile([128, PCH_GRP, 2, 128], f16)
                nc.sync.dma_start(ftg[:], feat_g[ng])
                for oct_ in range(1):                  # whole group batch
                    # cols 0-31: e logits; cols 32-63 (row 0): Z partials
                    ez_ps = ps_e.tile([128, 64], f32)
                    wT2 = pb_w.tile([128, 32], f16, tag="wT")
                    for blk in range(8):
                        # block = 2 pchunks = graphs (g0, g0+1)
                        pc0 = blk * 2
                        g0 = ng * PCH_GRP + pc0
                        fT_ps = ps_ft.tile([128, 512], f16)
                        for c in range(4):
                            nc.tensor.transpose(
                                fT_ps[:, ts(c, 128)],
                                ftg[:, pc0 + c // 2, c % 2, :],
                                ident_h[:])
                        fT_sb = pb_sb.tile([128, 512], f16, tag="fT")
                        nc.vector.tensor_copy(fT_sb[:], fT_ps[:])
                        uT_ps = ps_u.tile([128, 512], f32)
                        nc.tensor.matmul(uT_ps[:], Wu_h[:], fT_sb[:],
                                         start=True, stop=True)
                        sigT = pb_sb.tile([128, 512], f16, tag="sigT")
                        for gb in range(2):
                            nc.scalar.activation(
                                sigT[:, ts(gb, 256)],
                                uT_ps[:, ts(gb, 256)],
                                AF.Tanh, bias=vT_sb[:, g0 + gb:g0 + gb + 1],
                                scale=0.5)
                        for c in range(4):
                            nc.tensor.matmul(
                                ez_ps[:, blk * 4 + c:blk * 4 + c + 1],
                                sigT[:, ts(c, 128)], we_h[:],
                                start=True, stop=True)
                    # exp over 32 half-graph weight columns (8 blocks)
                    nc.scalar.activation(wT2[:], ez_ps[:, 0:32], AF.Exp,
                                         bias=c0b[:], scale=1.0)
                    mmz = nc.tensor.matmul(ez_ps[0:1, 32:64], ones_h[:],
                                           wT2[:], start=True, stop=True,
                                           skip_group_check=True)
                    mmz.is_weight_onezero = True
                    nc.vector.tensor_copy(z2_sb[:, ts(ng, 32)],
                                          ez_ps[0:1, 32:64])
                    for c32 in range(32):
                        pc = c32 // 2
                        g = ng * PCH_GRP + pc
                        nc.tensor.matmul(PZ[:, g:g + 1],
                                         ftg[:, pc, c32 % 2, :],
                                         wT2[:, c32:c32 + 1],
                                         start=(c32 % 2 == 0),
                                         stop=(c32 % 2 == 1),
                                         skip_group_check=True)

          # copy pooled results out of PSUM so those banks free up for the
          # tail
          poolRaw = consts.tile([128, g_core], f32, tag="poolRaw")
          nc.vector.tensor_copy(poolRaw[:], PZ[:])

        # ---------------- Tail: W_out + 1/z + output ----------------
        with tc.tile_pool(name="tail_sb", bufs=2) as tail_sb, \
             tc.tile_pool(name="tail_ps", bufs=1, space="PSUM") as tail_ps:
              # fold Z2 chunk pairs -> zrow [1, g_core]
              z2v = z2_sb[:].rearrange("o (g two) -> o g two", two=2)
              zrow = consts.tile([1, g_core], f32, tag="zrow")
              nc.vector.tensor_add(zrow[:].rearrange("o (g one) -> o g one", one=1),
                                   z2v[:, :, 0:1], z2v[:, :, 1:2])
              rz_row = consts.tile([1, g_core], f32, tag="rz_row")
              nc.vector.reciprocal(rz_row[:], zrow[:])

              # W_out folded with a;  c_out = b @ W_out broadcast to rows
              Wout_a = consts.tile([128, OUT_DIM], f32, tag="Wout_a")
              nc.vector.tensor_scalar_mul(Wout_a[:], Wout_sb[:], aT[:])
              co_ps = tail_ps.tile([128, 2], f32, tag="co")
              for h in range(2):
                  nc.tensor.matmul(co_ps[:, h:h + 1], Wout_sb[:, ts(h, 128)],
                                   bT[:], start=True, stop=True)
              co_sb = consts.tile([128, 2], f32, tag="co_sb")
              nc.vector.tensor_copy(co_sb[:], co_ps[:])
              cor_ps = tail_ps.tile([1, 2, 128], f32, tag="cor")
              for h in range(2):
                  nc.tensor.transpose(cor_ps[:, h, :], co_sb[:, h:h + 1],
                                      ident_f[:])
              co_row = consts.tile([1, 2, 128], f32, tag="co_row")
              nc.vector.tensor_copy(co_row[:], cor_ps[:])
              cob_ps = tail_ps.tile([128, 2, 128], f32, tag="cob")
              nc.tensor.matmul(cob_ps[:], ones_row[:],
                               co_row[:].rearrange("o h d -> o (h d)"),
                               start=True, stop=True)
              co_bc = consts.tile([128, 2, 128], f32, tag="co_bc")
              nc.vector.tensor_copy(co_bc[:], cob_ps[:])

              rstT_sb = []
              for h in range(2):
                  rp = tail_ps.tile([128, g_core], f32, tag="rstT")
                  nc.tensor.matmul(rp[:], Wout_a[:, ts(h, 128)], poolRaw[:],
                                   start=True, stop=True)
                  rs_sb = tail_sb.tile([128, g_core], f32, tag="rstT_sb")
                  nc.vector.tensor_copy(rs_sb[:], rp[:])
                  rstT_sb.append(rs_sb)
              rst_r = rst[:, :].rearrange("(gc p) o -> gc p o", p=128)
              for gc in range(g_core // 128):
                  rzT_ps = tail_ps.tile([128, 1], f32, tag="rzT")
                  nc.tensor.transpose(rzT_ps[:], rz_row[:, ts(gc, 128)],
                                      ident_f[0:1, 0:1])
                  rzT = tail_sb.tile([128, 1], f32, tag="rzT_sb")
                  nc.vector.tensor_copy(rzT[:], rzT_ps[:])
                  rt_ps = tail_ps.tile([128, 2, 128], f32, tag="rt")
                  for h in range(2):
                      nc.tensor.transpose(rt_ps[:, h, :],
                                          rstT_sb[h][:, ts(gc, 128)],
                                          ident_f[:])
                  rt_sb = tail_sb.tile([128, 2, 128], f32, tag="rt_sb")
                  nc.vector.tensor_scalar_mul(rt_sb[:], rt_ps[:], rzT[:])
                  nc.vector.tensor_add(rt_sb[:], rt_sb[:], co_bc[:])
                  nc.sync.dma_start(rst_r[gc],
                                    rt_sb[:].rearrange("p h o -> p (h o)"))


def run_cores(in_maps, n_cores, g_core, trace=False):
    import concourse.bass_utils as bass_utils
    nc = build_nc(n_cores, g_core)
    return bass_utils.run_bass_kernel_spmd(
        nc, in_maps, core_ids=list(range(n_cores)), trace=trace)


def make_in_maps(inputs):
    feat = np.ascontiguousarray(inputs["feat"], np.float32)
    last = np.asarray(inputs["last_nodes"]).astype(np.int64)
    flast_full = np.ascontiguousarray(feat[last])
    feat_h = feat.astype(np.float16)
    in_maps = []
    for d in range(N_CORES):
        in_maps.append({
            "feat": feat_h[d * N_CORE:(d + 1) * N_CORE],
            "flast": flast_full[d * G_CORE:(d + 1) * G_CORE],
            **{k: np.ascontiguousarray(inputs[k], np.float32)
               for k in ("W_u", "W_v", "b_v", "w_e", "W_out", "gamma",
                         "beta")}})
    return in_maps


def _numpy_fallback(feat, gamma, beta, W_u, W_v, b_v, w_e, W_out,
                    segment_ids, last_nodes):
    mean = feat.mean(0)
    var = ((feat - mean) ** 2).mean(0)
    x = (feat - mean) / np.sqrt(var + BN_EPS) * gamma + beta
    fu = x @ W_u
    fv = x[last_nodes] @ W_v + b_v
    e = (1.0 / (1.0 + np.exp(-(fu + fv[segment_ids]))) @ w_e)[:, 0]
    G = int(segment_ids.max()) + 1
    m = np.full(G, -np.inf, np.float32)
    np.maximum.at(m, segment_ids, e)
    ex = np.exp(e - m[segment_ids])
    z = np.zeros(G, np.float32)
    np.add.at(z, segment_ids, ex)
    alpha = ex / z[segment_ids]
    rstv = np.zeros((G, feat.shape[1]), np.float32)
    np.add.at(rstv, segment_ids, x * alpha[:, None])
    return (rstv @ W_out).astype(np.float32)


def kernel(**inputs):
    feat = np.asarray(inputs["feat"])
    seg = np.asarray(inputs["segment_ids"])
    last = np.asarray(inputs["last_nodes"])
    expected_seg = np.repeat(np.arange(NUM_GRAPHS, dtype=np.int64),
                             NODES_PER_GRAPH)
    if feat.shape != (N_TOTAL, IN_DIM) or \
            not np.array_equal(seg.astype(np.int64), expected_seg):
        return _numpy_fallback(
            np.asarray(inputs["feat"], np.float32),
            np.asarray(inputs["gamma"], np.float32),
            np.asarray(inputs["beta"], np.float32),
            np.asarray(inputs["W_u"], np.float32),
            np.asarray(inputs["W_v"], np.float32),
            np.asarray(inputs["b_v"], np.float32),
            np.asarray(inputs["w_e"], np.float32),
            np.asarray(inputs["W_out"], np.float32),
            seg.astype(np.int64), last.astype(np.int64))

    in_maps = make_in_maps(inputs)
    res = run_cores(in_maps, N_CORES, G_CORE)
    out = np.concatenate([res.results[d]["rst"] for d in range(N_CORES)],
                         axis=0)
    return out.astype(np.float32)
